# revision 8
# baseline (speedup 1.0000x reference)
import sys
from contextlib import ExitStack

import numpy as np

sys.path.insert(0, "/opt/trn_rl_repo")

import ml_dtypes  # noqa: E402
import concourse.bass as bass  # noqa: E402
import concourse.bacc as bacc  # noqa: E402
import concourse.mybir as mybir  # noqa: E402
import concourse.tile as tile  # noqa: E402
from concourse.bass_utils import run_bass_kernel_spmd  # noqa: E402

BF16 = ml_dtypes.bfloat16
F32 = mybir.dt.float32
BF = mybir.dt.bfloat16

ND, NN, NO, SD = 65536, 16, 16, 4
NCORES = 8
NDC = ND // NCORES          # 8192 rows per core
FC = 1024                   # row-chunk (free dim) per pipeline iteration
NCH = NDC // FC             # 8 chunks
G = NDC // 128              # 64 row-groups of 128

R_AGENT, R_OBS, D_ROBOT, B_GAMMA = 0.15, 0.5, 0.49, 0.05

_CACHED = {}


def _build_bass():
    nc = bacc.Bacc()
    dt = nc.dram_tensor
    io = dict(
        npk=dt("npk", [8, 8 * NDC], BF, kind="ExternalInput"),
        opk=dt("opk", [4, 8 * NDC], BF, kind="ExternalInput"),
        gT=dt("gT", [2, NDC], BF, kind="ExternalInput"),
        onesd=dt("onesd", [1, NDC], BF, kind="ExternalInput"),
        xb=dt("xb", [128, 34 * G], F32, kind="ExternalInput"),
        w_n1=dt("w_n1", [8, 128], BF, kind="ExternalInput"),
        w_o1=dt("w_o1", [4, 128], BF, kind="ExternalInput"),
        w_n2=dt("w_n2", [128, 128], BF, kind="ExternalInput"),
        w_o2=dt("w_o2", [128, 128], BF, kind="ExternalInput"),
        w_sn=dt("w_sn", [128, 64], BF, kind="ExternalInput"),
        w_so=dt("w_so", [128, 64], BF, kind="ExternalInput"),
        w_bn=dt("w_bn", [64, 64], BF, kind="ExternalInput"),
        w_bo=dt("w_bo", [64, 64], BF, kind="ExternalInput"),
        w_g=dt("w_g", [2, 64], BF, kind="ExternalInput"),
        w_z2=dt("w_z2", [64, 64], BF, kind="ExternalInput"),
        w_p3=dt("w_p3", [65, 2], BF, kind="ExternalInput"),
        b_h1n=dt("b_h1n", [128, 1], F32, kind="ExternalInput"),
        b_h1o=dt("b_h1o", [128, 1], F32, kind="ExternalInput"),
        b_h2n=dt("b_h2n", [128, 1], F32, kind="ExternalInput"),
        b_h2o=dt("b_h2o", [128, 1], F32, kind="ExternalInput"),
        b_sn=dt("b_sn", [64, 1], F32, kind="ExternalInput"),
        b_so=dt("b_so", [64, 1], F32, kind="ExternalInput"),
        b_z1=dt("b_z1", [64, 1], F32, kind="ExternalInput"),
        b_z2=dt("b_z2", [64, 1], F32, kind="ExternalInput"),
        y=dt("y", [128, 16 * NCH], F32, kind="ExternalOutput"),
    )

    AF = mybir.ActivationFunctionType
    OP = mybir.AluOpType

    with ExitStack() as ctx:
        tc = ctx.enter_context(tile.TileContext(nc))
        wp = ctx.enter_context(tc.tile_pool(name="weights", bufs=1))
        bp = ctx.enter_context(tc.tile_pool(name="barrier", bufs=1))
        ip = ctx.enter_context(tc.tile_pool(name="inputs", bufs=2))
        sp = ctx.enter_context(tc.tile_pool(name="acts", bufs=1))
        tp = ctx.enter_context(tc.tile_pool(name="tail", bufs=2))
        psmm = ctx.enter_context(
            tc.tile_pool(name="psmm", bufs=2, space=bass.MemorySpace.PSUM))
        pss = ctx.enter_context(
            tc.tile_pool(name="pss", bufs=2, space=bass.MemorySpace.PSUM))
        psh = ctx.enter_context(
            tc.tile_pool(name="psh", bufs=2, space=bass.MemorySpace.PSUM))

        # ---- load weights/biases ----
        W = {}
        for name, shape, d in [
            ("w_n1", [8, 128], BF), ("w_o1", [4, 128], BF),
            ("w_n2", [128, 128], BF), ("w_o2", [128, 128], BF),
            ("w_sn", [128, 64], BF), ("w_so", [128, 64], BF),
            ("w_bn", [64, 64], BF), ("w_bo", [64, 64], BF),
            ("w_g", [2, 64], BF), ("w_z2", [64, 64], BF), ("w_p3", [65, 2], BF),
            ("b_h1n", [128, 1], F32), ("b_h1o", [128, 1], F32),
            ("b_h2n", [128, 1], F32), ("b_h2o", [128, 1], F32),
            ("b_sn", [64, 1], F32), ("b_so", [64, 1], F32),
            ("b_z1", [64, 1], F32), ("b_z2", [64, 1], F32),
        ]:
            t = wp.tile(shape, d, tag=name)
            nc.gpsimd.dma_start(t[:], io[name][:])
            W[name] = t

        # ---- barrier forces, full core at once, row-major [128, j, g] ----
        xbs = bp.tile([128, 34 * G], F32, tag="xbs")
        nc.gpsimd.dma_start(xbs[:], io["xb"][:])

        # Prime VectorE's vector-clock past every input DMA so real DVE ops
        # never need a DMA wait (TensorTensor ISA has a single wait slot).
        dmy = bp.tile([128, 1], F32, tag="dmy")
        for t_ in list(W.values()) + [xbs]:
            p_ = t_.shape[0]
            nc.vector.tensor_copy(dmy[0:p_, :], t_[:, 0:1])
        sq = bp.tile([128, 34 * G], F32, tag="sq")
        nc.scalar.activation(sq[:], xbs[:], AF.Square)
        d2 = bp.tile([128, 18 * G], F32, tag="d2")
        nc.vector.tensor_add(d2[:, 0:16 * G], sq[:, 0:16 * G], sq[:, 16 * G:32 * G])
        nc.vector.tensor_add(d2[:, 16 * G:17 * G], sq[:, 32 * G:33 * G],
                             sq[:, 33 * G:34 * G])
        nc.vector.tensor_tensor(d2[:, 17 * G:18 * G], sq[:, 32 * G:33 * G],
                                sq[:, 33 * G:34 * G], op=OP.max)
        rt = bp.tile([128, 18 * G], F32, tag="rt")
        nc.scalar.activation(rt[:], d2[:], AF.Sqrt)
        # ihb = 1/(40*dist - 19.6) = 0.025/(dist - 0.49)
        hbp = bp.tile([128, 16 * G], F32, tag="hbp")
        nc.vector.tensor_scalar(hbp[:], rt[:, 0:16 * G], 40.0, -19.6,
                                op0=OP.mult, op1=OP.add)
        ihb = bp.tile([128, 16 * G], F32, tag="ihb")
        nc.vector.reciprocal(ihb[:], hbp[:])
        # iho = 1/(2.5*dn - 1.25*q - 0.375) = 0.4/ho,  q = dn/m
        im = bp.tile([128, G], F32, tag="im")
        nc.vector.reciprocal(im[:], rt[:, 17 * G:18 * G])
        q = bp.tile([128, G], F32, tag="q")
        nc.vector.tensor_mul(q[:], rt[:, 16 * G:17 * G], im[:])
        w1 = bp.tile([128, G], F32, tag="w1")
        nc.vector.tensor_scalar(w1[:], rt[:, 16 * G:17 * G], 2.5, -0.375,
                                op0=OP.mult, op1=OP.add)
        w2 = bp.tile([128, G], F32, tag="w2")
        nc.vector.scalar_tensor_tensor(w2[:], q[:], -1.25, w1[:],
                                       op0=OP.mult, op1=OP.add)
        iho = bp.tile([128, G], F32, tag="iho")
        nc.vector.reciprocal(iho[:], w2[:])
        # cp = p * ihb  (px pre-negated on host; ihb carries 0.05/2)
        cpx = bp.tile([128, 16 * G], F32, tag="cpx")
        nc.vector.tensor_mul(cpx[:], xbs[:, 0:16 * G], ihb[:])
        cpy = bp.tile([128, 16 * G], F32, tag="cpy")
        nc.vector.tensor_mul(cpy[:], xbs[:, 16 * G:32 * G], ihb[:])
        fnx = bp.tile([128, G], F32, tag="fnx")
        nc.vector.tensor_reduce(
            fnx[:], cpx[:].rearrange("p (j g) -> p g j", j=16),
            axis=mybir.AxisListType.X, op=OP.add)
        fny = bp.tile([128, G], F32, tag="fny")
        nc.vector.tensor_reduce(
            fny[:], cpy[:].rearrange("p (j g) -> p g j", j=16),
            axis=mybir.AxisListType.X, op=OP.add)
        fox = bp.tile([128, G], F32, tag="fox")
        nc.vector.tensor_mul(fox[:], xbs[:, 32 * G:33 * G], iho[:])
        foy = bp.tile([128, G], F32, tag="foy")
        nc.vector.tensor_mul(foy[:], xbs[:, 33 * G:34 * G], iho[:])
        fh = bp.tile([128, 2 * G], F32, tag="fh")
        fh3 = fh[:].rearrange("p (g c) -> p g c", c=2)
        nc.vector.tensor_add(fh3[:, :, 0], fnx[:], fox[:])
        nc.vector.tensor_add(fh3[:, :, 1], fny[:], foy[:])

        npk3 = io["npk"][:].rearrange("k (t r) -> k t r", t=8)
        opk3 = io["opk"][:].rearrange("k (t r) -> k t r", t=8)

        def evict_act(dst, src, bias):
            nc.scalar.activation(dst, src, AF.Relu, bias=bias)

        def evict_dve(dst, src, bias):
            nc.vector.tensor_scalar(dst, src, bias, 0.0, op0=OP.add, op1=OP.max)

        for c in range(NCH):
            n1r = ip.tile([8, 8 * FC], BF, tag="n1r")
            nc.gpsimd.dma_start(n1r[:], npk3[:, :, c * FC:(c + 1) * FC])
            n1r3 = n1r[:].rearrange("k (t r) -> k t r", t=8)
            o1r = ip.tile([4, 8 * FC], BF, tag="o1r")
            nc.gpsimd.dma_start(o1r[:], opk3[:, :, c * FC:(c + 1) * FC])
            o1r3 = o1r[:].rearrange("k (t r) -> k t r", t=8)
            gsb = ip.tile([2, FC], BF, tag="gsb")
            nc.gpsimd.dma_start(gsb[:], io["gT"][:, c * FC:(c + 1) * FC])

            h1n = sp.tile([128, 8 * FC], BF, tag="h1n")
            h1n3 = h1n[:].rearrange("p (t r) -> p t r", t=8)
            h1o = sp.tile([128, 8 * FC], BF, tag="h1o")
            h1o3 = h1o[:].rearrange("p (t r) -> p t r", t=8)
            for t in range(8):
                ps = psmm.tile([128, FC], F32, tag="mm")
                for h in range(FC // 512):
                    nc.tensor.matmul(ps[:, h * 512:(h + 1) * 512], W["w_n1"][:],
                                     n1r3[:, t, h * 512:(h + 1) * 512],
                                     start=True, stop=True)
                evict_act(h1n3[:, t, :], ps[:], W["b_h1n"][:])
                ps = psmm.tile([128, FC], F32, tag="mm")
                for h in range(FC // 512):
                    nc.tensor.matmul(ps[:, h * 512:(h + 1) * 512], W["w_o1"][:],
                                     o1r3[:, t, h * 512:(h + 1) * 512],
                                     start=True, stop=True)
                evict_dve(h1o3[:, t, :], ps[:], W["b_h1o"][:])

            h2n = sp.tile([128, 8 * FC], BF, tag="h2n")
            h2n3 = h2n[:].rearrange("p (t r) -> p t r", t=8)
            h2o = sp.tile([128, 8 * FC], BF, tag="h2o")
            h2o3 = h2o[:].rearrange("p (t r) -> p t r", t=8)
            for t in range(8):
                ps = psmm.tile([128, FC], F32, tag="mm")
                for h in range(FC // 512):
                    nc.tensor.matmul(ps[:, h * 512:(h + 1) * 512], W["w_n2"][:],
                                     h1n3[:, t, h * 512:(h + 1) * 512],
                                     start=True, stop=True)
                evict_act(h2n3[:, t, :], ps[:], W["b_h2n"][:])
                ps = psmm.tile([128, FC], F32, tag="mm")
                for h in range(FC // 512):
                    nc.tensor.matmul(ps[:, h * 512:(h + 1) * 512], W["w_o2"][:],
                                     h1o3[:, t, h * 512:(h + 1) * 512],
                                     start=True, stop=True)
                evict_dve(h2o3[:, t, :], ps[:], W["b_h2o"][:])

            sn = tp.tile([64, FC], BF, tag="sn")
            so = tp.tile([64, FC], BF, tag="so")
            for h in range(FC // 512):
                ps = pss.tile([64, 512], F32, tag="s")
                for t in range(8):
                    nc.tensor.matmul(ps[:], W["w_sn"][:],
                                     h2n3[:, t, h * 512:(h + 1) * 512],
                                     start=(t == 0), stop=(t == 7))
                evict_act(sn[:, h * 512:(h + 1) * 512], ps[:], W["b_sn"][:])
                ps = pss.tile([64, 512], F32, tag="s")
                for t in range(8):
                    nc.tensor.matmul(ps[:], W["w_so"][:],
                                     h2o3[:, t, h * 512:(h + 1) * 512],
                                     start=(t == 0), stop=(t == 7))
                evict_act(so[:, h * 512:(h + 1) * 512], ps[:], W["b_so"][:])

            z2e = tp.tile([65, FC], BF, tag="z2e")
            nc.gpsimd.dma_start(z2e[64:65, :], io["onesd"][:, c * FC:(c + 1) * FC])
            z1 = tp.tile([64, FC], BF, tag="z1")
            for h in range(FC // 512):
                sl = slice(h * 512, (h + 1) * 512)
                ps = pss.tile([64, 512], F32, tag="s")
                nc.tensor.matmul(ps[:], W["w_bn"][:], sn[:, sl], start=True, stop=False)
                nc.tensor.matmul(ps[:], W["w_bo"][:], so[:, sl], start=False, stop=False)
                nc.tensor.matmul(ps[:], W["w_g"][:], gsb[:, sl], start=False, stop=True)
                evict_act(z1[:, sl], ps[:], W["b_z1"][:])
                ps = pss.tile([64, 512], F32, tag="s")
                nc.tensor.matmul(ps[:], W["w_z2"][:], z1[:, sl], start=True, stop=True)
                evict_act(z2e[0:64, sl], ps[:], W["b_z2"][:])

            hps = psh.tile([128, 16], F32, tag="hps")
            for j in range(8):
                nc.tensor.matmul(hps[:, j * 2:j * 2 + 2],
                                 z2e[:, j * 128:(j + 1) * 128], W["w_p3"][:],
                                 start=True, stop=True)
            trm = tp.tile([128, 16], F32, tag="trm")
            nc.scalar.activation(trm[:], hps[:], AF.Tanh)
            a2 = tp.tile([128, 16], F32, tag="a2")
            nc.vector.tensor_add(a2[:], trm[:], fh[:, c * 16:(c + 1) * 16])
            ot = tp.tile([128, 16], F32, tag="ot")
            nc.scalar.activation(ot[:], a2[:], AF.Tanh, scale=2.0)
            ysb = tp.tile([128, 16], F32, tag="ysb")
            nc.vector.tensor_scalar_mul(ysb[:], ot[:], 2.0)
            nc.sync.dma_start(io["y"][:, c * 16:(c + 1) * 16], ysb[:])

    nc.compile()
    return nc


def _prep_consts(params):
    g = lambda a: np.asarray(a, dtype=np.float32)
    (W1n, b1n), (W2n, b2n), (W3n, b3n) = [(g(w), g(b)) for w, b in params["phi_n"]]
    (Wrn1, brn1), (Wrn2, brn2) = [(g(w), g(b)) for w, b in params["rho_n"]]
    (W1o, b1o), (W2o, b2o), (W3o, b3o) = [(g(w), g(b)) for w, b in params["phi_o"]]
    (Wro1, bro1), (Wro2, bro2) = [(g(w), g(b)) for w, b in params["rho_o"]]
    (Wp1, bp1), (Wp2, bp2), (Wp3, bp3) = [(g(w), g(b)) for w, b in params["psi"]]

    Wtn = Wrn1 @ W3n
    btn = Wrn1 @ (NN * b3n) + brn1
    Wto = Wro1 @ W3o
    bto = Wro1 @ (NO * b3o) + bro1
    An, Ao, Ag = Wp1[:, 0:8], Wp1[:, 8:16], Wp1[:, 16:18]
    Bn = An @ Wrn2
    Bo = Ao @ Wro2
    bz1 = An @ brn2 + Ao @ bro2 + bp1

    def diag2(Wt):  # [k, m] -> [2k, 2m] block-diag
        k, m = Wt.shape
        out = np.zeros((2 * k, 2 * m), np.float32)
        out[0:k, 0:m] = Wt
        out[k:, m:] = Wt
        return out

    c = {}
    c["w_n1"] = diag2(W1n.T).astype(BF16)            # [8, 128]
    c["w_o1"] = diag2(W1o.T).astype(BF16)            # [4, 128]
    c["w_n2"] = diag2(W2n.T).astype(BF16)            # [128, 128]
    c["w_o2"] = diag2(W2o.T).astype(BF16)
    c["w_sn"] = np.vstack([Wtn.T, Wtn.T]).astype(BF16)   # [128, 64]
    c["w_so"] = np.vstack([Wto.T, Wto.T]).astype(BF16)
    c["w_bn"] = Bn.T.astype(BF16)
    c["w_bo"] = Bo.T.astype(BF16)
    c["w_g"] = Ag.T.astype(BF16)                     # [2, 64]
    c["w_z2"] = Wp2.T.astype(BF16)
    c["w_p3"] = np.vstack([Wp3.T, bp3[None, :]]).astype(BF16)  # [65, 2]
    c["b_h1n"] = np.concatenate([b1n, b1n])[:, None]
    c["b_h1o"] = np.concatenate([b1o, b1o])[:, None]
    c["b_h2n"] = np.concatenate([b2n, b2n])[:, None]
    c["b_h2o"] = np.concatenate([b2o, b2o])[:, None]
    c["b_sn"] = btn[:, None]
    c["b_so"] = bto[:, None]
    c["b_z1"] = bz1[:, None]
    c["b_z2"] = bp2[:, None]
    for k in list(c):
        if c[k].dtype == np.float32:
            c[k] = np.ascontiguousarray(c[k], dtype=np.float32)
    return c


def _prep_core_inputs(xc):
    # xc: [NDC, 101] fp32
    out = {}
    neigh = xc[:, 5:69]
    out["npk"] = np.ascontiguousarray(
        neigh.reshape(NDC, 8, 8).transpose(2, 1, 0).reshape(8, 8 * NDC)
    ).astype(BF16)
    obs = xc[:, 69:101]
    out["opk"] = np.ascontiguousarray(
        obs.reshape(NDC, 8, 4).transpose(2, 1, 0).reshape(4, 8 * NDC)
    ).astype(BF16)
    out["gT"] = np.ascontiguousarray(xc[:, 1:3].T).astype(BF16)
    out["onesd"] = np.ones((1, NDC), dtype=BF16)
    xv = xc.reshape(G, 128, 101)
    xb = np.empty((128, 34, G), np.float32)
    xb[:, 0:16, :] = -xv[:, :, 5:69:4].transpose(1, 2, 0)
    xb[:, 16:32, :] = -xv[:, :, 6:69:4].transpose(1, 2, 0)
    xb[:, 32, :] = xv[:, :, 69].T
    xb[:, 33, :] = xv[:, :, 70].T
    out["xb"] = np.ascontiguousarray(xb.reshape(128, 34 * G), dtype=np.float32)
    return out


def kernel(x, params):
    x = np.asarray(x, dtype=np.float32)
    if "nc" not in _CACHED:
        _CACHED["nc"] = _build_bass()
    nc = _CACHED["nc"]
    consts = _prep_consts(params)
    in_maps = []
    for core in range(NCORES):
        m = dict(consts)
        m.update(_prep_core_inputs(x[core * NDC:(core + 1) * NDC]))
        in_maps.append(m)
    res = run_bass_kernel_spmd(nc, in_maps, core_ids=list(range(NCORES)))
    outs = []
    for core in range(NCORES):
        y = res.results[core]["y"]
        outs.append(
            y.reshape(128, NCH, 8, 2).transpose(1, 2, 0, 3).reshape(NDC, 2))
    return np.ascontiguousarray(np.concatenate(outs, axis=0), dtype=np.float32)
